# revision 12
# baseline (speedup 1.0000x reference)
"""TRN2 Bass/Tile kernel for nn_Loss_58317065945194.

Loss: per-sample EMD with r=2 over C=10 channels:
    d = p - q                       # [B, C]
    S = cumsum(d, axis=1)           # per-sample prefix sums
    per_sample = sqrt(mean(S**2))   # [B]
    out = mean(per_sample)          # scalar

Strategy (pure data parallel, 8 cores):
  - Shard B across 8 cores. Host prep computes the elementwise
    difference d = p - q in fp32, casts to fp16 and lays each core's
    shard out channel-major: partition row = [C=10 planes][W=2048
    samples], so every on-device op is a 2D unit-stride fp16 access
    pattern (tensor_tensor runs in 2x_1p mode) and input DMA is halved.
  - Per channel plane c (pipelined at plane granularity):
      * DMA d_c ([128, 2048] fp16)
      * Vector: S_c = d_c + S_{c-1} in place (chained prefix adds --
        no scan, no boundary fixup)
      * Scalar ACT: sq_c = S_c^2
      * Vector: U += sq_c (fp16 accumulator)
  - ACT: loss = sqrt(U / C), accum_out -> batch partial [128, 1].
  - Host sums the 8 cores' partials and divides by B.
"""

import sys

import numpy as np

if "/opt/trn_rl_repo" not in sys.path:
    sys.path.insert(0, "/opt/trn_rl_repo")

N_CORES = 8
B, C = 2097152, 10
BS = B // N_CORES        # samples per core shard (262144)
P = 128                  # SBUF partitions
W = BS // P              # samples per partition = plane width (2048)
FPP = W * C              # elems per partition (20480)

_cache = {}


def _build_program():
    import concourse.tile as tile
    from concourse import bacc, mybir

    f32, f16 = mybir.dt.float32, mybir.dt.float16
    Alu = mybir.AluOpType
    Act = mybir.ActivationFunctionType

    nc = bacc.Bacc(
        "TRN2", target_bir_lowering=False, debug=False, num_devices=N_CORES
    )
    d_d = nc.dram_tensor("d", [P, FPP], f16, kind="ExternalInput").ap()
    o_d = nc.dram_tensor("partial", [P, 2], f32, kind="ExternalOutput").ap()

    with tile.TileContext(nc) as tc:
        with (
            tc.tile_pool(name="io", bufs=1) as io,
            tc.tile_pool(name="work", bufs=1) as work,
            tc.tile_pool(name="small", bufs=1) as small,
        ):
            S = [io.tile([P, W], f16, tag=f"S{c}", name=f"S{c}") for c in range(C)]
            SQ = [work.tile([P, W], f16, tag=f"sq{c}", name=f"sq{c}") for c in range(C)]
            U = small.tile([P, W], f16, tag="U")
            acc = small.tile([P, 2], f32, tag="acc")

            # preload both ACT function tables off the critical path
            warm = small.tile([P, 1], f32, tag="warm")
            nc.scalar.activation(warm[:], warm[:], Act.Square)
            nc.scalar.activation(warm[:], warm[:], Act.Sqrt)

            # d planes stream in; early planes split so compute starts sooner
            splits = {}
            for c in range(C):
                n = splits.get(c, 1)
                step = W // n
                for k in range(n):
                    sl = slice(c * W + k * step, c * W + (k + 1) * step)
                    tl = slice(k * step, (k + 1) * step)
                    nc.sync.dma_start(S[c][:, tl], d_d[:, sl])

            H = W // 2
            Ug = small.tile([P, W], f16, tag="Ug")

            def chain(c):
                nc.vector.tensor_tensor(S[c][:], S[c][:], S[c - 1][:], Alu.add)

            def square(c):
                # planes 1-2 squared on Vector (it idles early, DMA-paced);
                # the rest on ACT
                if c in (1, 2):
                    nc.vector.tensor_tensor(SQ[c][:], S[c][:], S[c][:], Alu.mult)
                else:
                    nc.scalar.activation(SQ[c][:], S[c][:], Act.Square)

            def uadd_v(c):
                # odd planes accumulate into U on Vector
                if c == 3:
                    nc.vector.tensor_tensor(U[:], SQ[1][:], SQ[3][:], Alu.add)
                else:
                    nc.vector.tensor_tensor(U[:], U[:], SQ[c][:], Alu.add)

            def uadd_g(c):
                # even planes accumulate into Ug via DMA-compute (SWDGE):
                # data is moved/added by the DMA engines, not the DSPs
                if c == 0:
                    nc.sync.dma_start(Ug[:], SQ[0][:])
                else:
                    nc.gpsimd.dma_start(Ug[:], SQ[c][:], accum_op=Alu.add)

            square(0)
            uadd_g(0)
            for c in range(1, C):
                chain(c)
                square(c)
                if c >= 2 and c % 2 == 0:
                    uadd_g(c)
                if c >= 3 and c % 2 == 1:
                    uadd_v(c)

            # merge accumulators, then loss = sqrt(U / C) per half with
            # batch-sum accumulation
            lt = small.tile([P, W], f32, tag="loss")
            for h in range(2):
                hs = slice(h * H, (h + 1) * H)
                nc.vector.tensor_tensor(U[:, hs], U[:, hs], Ug[:, hs], Alu.add)
                nc.scalar.activation(
                    lt[:, hs], U[:, hs], Act.Sqrt, scale=1.0 / C,
                    accum_out=acc[:, h : h + 1],
                )
            nc.sync.dma_start(o_d[:], acc[:])
    nc.compile()
    return nc


def _make_in_maps(p, q):
    p = np.asarray(p, dtype=np.float32).reshape(B, C)
    q = np.asarray(q, dtype=np.float32).reshape(B, C)
    d = (p - q).astype(np.float16)

    def prep(i):
        sh = d[i * BS : (i + 1) * BS].reshape(P, W, C)
        return np.ascontiguousarray(sh.transpose(0, 2, 1)).reshape(P, FPP)

    return [{"d": prep(i)} for i in range(N_CORES)]


def kernel(p, q, r):
    assert int(r) == 2, f"kernel specialized for r=2, got {r}"
    if "nc" not in _cache:
        _cache["nc"] = _build_program()
    nc = _cache["nc"]

    in_maps = _make_in_maps(p, q)

    from concourse.bass_utils import run_bass_kernel_spmd

    res = run_bass_kernel_spmd(nc, in_maps, list(range(N_CORES)))
    total = 0.0
    for r_ in res.results:
        total += r_["partial"].astype(np.float64).sum()
    return np.float32(total / B)


# revision 13
# speedup vs baseline: 1.2929x; 1.2929x over previous
"""TRN2 Bass/Tile kernel for nn_Loss_58317065945194.

Loss: per-sample EMD with r=2 over C=10 channels:
    d = p - q                       # [B, C]
    S = cumsum(d, axis=1)           # per-sample prefix sums
    per_sample = sqrt(mean(S**2))   # [B]
    out = mean(per_sample)          # scalar

Strategy (pure data parallel, 8 cores):
  - Shard B across 8 cores. Host prep computes the elementwise
    difference d = p - q in fp32, casts to fp16 and lays each core's
    shard out channel-major: partition row = [C=10 planes][W=2048
    samples], so every on-device op is a 2D unit-stride fp16 access
    pattern (tensor_tensor runs in 2x_1p mode) and input DMA is halved.
  - Per channel plane c (pipelined at plane granularity):
      * DMA d_c ([128, 2048] fp16)
      * Vector: S_c = d_c + S_{c-1} in place (chained prefix adds --
        no scan, no boundary fixup)
      * Scalar ACT: sq_c = S_c^2
      * Vector: U += sq_c (fp16 accumulator)
  - ACT: loss = sqrt(U / C), accum_out -> batch partial [128, 1].
  - Host sums the 8 cores' partials and divides by B.
"""

import sys

import numpy as np

if "/opt/trn_rl_repo" not in sys.path:
    sys.path.insert(0, "/opt/trn_rl_repo")

N_CORES = 8
B, C = 2097152, 10
BS = B // N_CORES        # samples per core shard (262144)
P = 128                  # SBUF partitions
W = BS // P              # samples per partition = plane width (2048)
FPP = W * C              # elems per partition (20480)

_cache = {}


def _build_program():
    import concourse.tile as tile
    from concourse import bacc, mybir

    f32, f16 = mybir.dt.float32, mybir.dt.float16
    Alu = mybir.AluOpType
    Act = mybir.ActivationFunctionType

    nc = bacc.Bacc(
        "TRN2", target_bir_lowering=False, debug=False, num_devices=N_CORES
    )
    d_d = nc.dram_tensor("d", [P, FPP], f16, kind="ExternalInput").ap()
    o_d = nc.dram_tensor("partial", [P, 2], f32, kind="ExternalOutput").ap()

    with tile.TileContext(nc) as tc:
        with (
            tc.tile_pool(name="io", bufs=1) as io,
            tc.tile_pool(name="work", bufs=1) as work,
            tc.tile_pool(name="small", bufs=1) as small,
        ):
            S = [io.tile([P, W], f16, tag=f"S{c}", name=f"S{c}") for c in range(C)]
            SQ = [work.tile([P, W], f16, tag=f"sq{c}", name=f"sq{c}") for c in range(C)]
            U = small.tile([P, W], f16, tag="U")
            acc = small.tile([P, 2], f32, tag="acc")

            # preload both ACT function tables off the critical path
            warm = small.tile([P, 1], f32, tag="warm")
            nc.scalar.activation(warm[:], warm[:], Act.Square)
            nc.scalar.activation(warm[:], warm[:], Act.Sqrt)

            # d planes stream in; early planes split so compute starts sooner
            splits = {}
            for c in range(C):
                n = splits.get(c, 1)
                step = W // n
                for k in range(n):
                    sl = slice(c * W + k * step, c * W + (k + 1) * step)
                    tl = slice(k * step, (k + 1) * step)
                    nc.sync.dma_start(S[c][:, tl], d_d[:, sl])

            H = W // 2

            def chain(c):
                # S_c += S_{c-1} (prefix chain)
                nc.vector.tensor_tensor(S[c][:], S[c][:], S[c - 1][:], Alu.add)

            def square(c):
                if c < C - 1:
                    nc.scalar.activation(SQ[c][:], S[c][:], Act.Square)
                else:
                    for h in range(2):
                        hs = slice(h * H, (h + 1) * H)
                        nc.scalar.activation(
                            SQ[c][:, hs], S[c][:, hs], Act.Square
                        )

            def uadd(c):
                # U accumulation (full planes; last plane in halves for tail)
                halves = 2 if c == C - 1 else 1
                for h in range(halves):
                    hs = slice(h * (W // halves), (h + 1) * (W // halves))
                    if c == 1:
                        nc.vector.tensor_tensor(
                            U[:, hs], SQ[0][:, hs], SQ[1][:, hs], Alu.add
                        )
                    else:
                        nc.vector.tensor_tensor(
                            U[:, hs], U[:, hs], SQ[c][:, hs], Alu.add
                        )

            # software-pipelined issue order: V's in-order queue must not
            # stall on a U-add whose square isn't ready, so U lags by 2
            square(0)
            for c in range(1, C):
                chain(c)
                square(c)
                if c >= 3:
                    uadd(c - 2)
            uadd(C - 2)
            uadd(C - 1)

            # loss[g] = sqrt(U[g] / C); acc[:, h] = sum_g loss[g] per half
            lt = small.tile([P, W], f32, tag="loss")
            for h in range(2):
                hs = slice(h * H, (h + 1) * H)
                nc.scalar.activation(
                    lt[:, hs], U[:, hs], Act.Sqrt, scale=1.0 / C,
                    accum_out=acc[:, h : h + 1],
                )
            nc.sync.dma_start(o_d[:], acc[:])
    nc.compile()
    return nc


def _make_in_maps(p, q):
    p = np.asarray(p, dtype=np.float32).reshape(B, C)
    q = np.asarray(q, dtype=np.float32).reshape(B, C)
    d = (p - q).astype(np.float16)

    def prep(i):
        sh = d[i * BS : (i + 1) * BS].reshape(P, W, C)
        return np.ascontiguousarray(sh.transpose(0, 2, 1)).reshape(P, FPP)

    return [{"d": prep(i)} for i in range(N_CORES)]


def kernel(p, q, r):
    assert int(r) == 2, f"kernel specialized for r=2, got {r}"
    if "nc" not in _cache:
        _cache["nc"] = _build_program()
    nc = _cache["nc"]

    in_maps = _make_in_maps(p, q)

    from concourse.bass_utils import run_bass_kernel_spmd

    res = run_bass_kernel_spmd(nc, in_maps, list(range(N_CORES)))
    total = 0.0
    for r_ in res.results:
        total += r_["partial"].astype(np.float64).sum()
    return np.float32(total / B)
